# revision 1
# baseline (speedup 1.0000x reference)
"""Trainium2 Bass kernel for nn_IrBinaryLinear (binarized linear layer).

Reference computation (fp32):
    w  = weight - mean(weight, axis=-1, keepdims=True)       # [out, in]
    s  = mean(|w|, axis=-1, keepdims=True)                   # [out, 1]
    wb = sign(w) * s                                         # [out, in]
    y  = x @ wb.T + bias                                     # [B, S, out]

Sharding: tensor-parallel over weight rows (out_features) across 8 cores.
Each core binarizes its own 512-row weight shard on device, transposes it
on the PE array, and streams the (replicated) activations through XBAR
transpose DMA loads to feed the tensor engine with bf16 matmuls (the
binarized weights are exactly +/-scale, which bf16 represents with only a
2^-9 relative rounding of the scale; x is bf16-rounded). PSUM accumulates
in fp32 and the bias is added on the vector engine before the fp32 store.
"""

import numpy as np
import ml_dtypes

import concourse.bass as bass
import concourse.tile as tile
from concourse import bacc, mybir
from concourse.bass_utils import run_bass_kernel_spmd
from concourse.masks import make_identity

F32 = mybir.dt.float32
BF16 = mybir.dt.bfloat16

N_CORES = 8
B, S, DIN, DOUT = 4, 2048, 4096, 4096
TOK = B * S                    # 8192 tokens
OSH = DOUT // N_CORES          # 512 output rows per core
KC = DIN // 128                # 32 contraction chunks
TOKG = 512                     # tokens per XBAR-load group
RT = OSH // 128                # weight row tiles per core


def build_kernel_nc(tok=TOK, osh=OSH, tokg=TOKG, n_cores=N_CORES):
    """Build + compile the per-core Bass program (SPMD: same on all cores)."""
    kc = KC
    rt_n = osh // 128
    ntg = tok // tokg

    nc = bacc.Bacc("TRN2", target_bir_lowering=False, debug=False,
                   num_devices=n_cores)
    xb_d = nc.dram_tensor("xb", [kc, tok, 128], BF16, kind="ExternalInput")
    w_d = nc.dram_tensor("w", [osh, DIN], F32, kind="ExternalInput")
    bias_d = nc.dram_tensor("bias", [osh], F32, kind="ExternalInput")
    out_d = nc.dram_tensor("out", [tok, osh], F32, kind="ExternalOutput")

    with tile.TileContext(nc) as tc:
        _body(tc, nc, xb_d.ap(), w_d.ap(), bias_d.ap(), out_d.ap(),
              tok=tok, osh=osh, tokg=tokg, kc=kc, rt_n=rt_n, ntg=ntg)

    nc.compile()
    return nc


def _body(tc, nc, xb, w, bias, out, *, tok, osh, tokg, kc, rt_n, ntg):
    with (
        tc.tile_pool(name="consts", bufs=1) as consts,
        tc.tile_pool(name="wld", bufs=2) as wld,
        tc.tile_pool(name="wsg", bufs=2) as wsg,
        tc.tile_pool(name="wst", bufs=8) as wst,
        tc.tile_pool(name="wbtp", bufs=1) as wbtp,
        tc.tile_pool(name="tps", bufs=2, space="PSUM") as tps,
        tc.tile_pool(name="xtp", bufs=2) as xtp,
        tc.tile_pool(name="ops", bufs=4, space="PSUM") as ops,
        tc.tile_pool(name="otp", bufs=3) as otp,
    ):
        ident = consts.tile([128, 128], BF16)
        make_identity(nc, ident)

        # bias broadcast to all 128 partitions: [osh] -> [128, osh]
        bias_bc = consts.tile([128, osh], F32)
        bias_bcast_ap = bass.AP(
            tensor=bias.tensor, offset=bias.offset,
            ap=[[0, 128]] + list(bias.ap),
        )
        nc.gpsimd.dma_start(out=bias_bc, in_=bias_bcast_ap)

        # Binarized transposed weights, resident: [128(i), kc, osh] bf16
        wbT = wbtp.tile([128, kc, osh], BF16)

        for rt in range(rt_n):
            wt = wld.tile([128, DIN], F32)
            nc.sync.dma_start(out=wt, in_=w[rt * 128:(rt + 1) * 128, :])

            rs = wst.tile([128, 1], F32)
            nc.vector.tensor_reduce(out=rs, in_=wt, axis=mybir.AxisListType.X,
                                    op=mybir.AluOpType.add)
            nmean = wst.tile([128, 1], F32)
            nc.vector.tensor_scalar_mul(nmean, rs, -1.0 / DIN)

            # sgn = sign(w - mean)  (exact +/-1, bf16)
            sgn = wsg.tile([128, DIN], BF16)
            nc.scalar.sign(out=sgn, in_=wt, bias=nmean)

            # |w - mean| = (w + nmean) * sgn ; row-sum into asum
            asum = wst.tile([128, 1], F32)
            nc.vector.scalar_tensor_tensor(
                out=wt, in0=wt, scalar=nmean, in1=sgn,
                op0=mybir.AluOpType.add, op1=mybir.AluOpType.mult,
                accum_out=asum,
            )
            scale = wst.tile([128, 1], F32)
            nc.vector.tensor_scalar_mul(scale, asum, 1.0 / DIN)

            # wb row tile = sgn * scale (in place, bf16)
            nc.vector.tensor_scalar_mul(sgn, sgn, scale)

            # transpose [128(o), 128(i)] chunks onto PE -> wbT[:, c, o-range]
            for c in range(kc):
                pt = tps.tile([128, 128], BF16)
                nc.tensor.transpose(pt, sgn[:, c * 128:(c + 1) * 128], ident)
                nc.vector.tensor_copy(
                    out=wbT[:, c, rt * 128:(rt + 1) * 128], in_=pt)

        # main loop: stream x through XBAR transpose, matmul, bias, store
        for g in range(ntg):
            xt = xtp.tile([128, kc, tokg], BF16)
            for c in range(kc):
                nc.sync.dma_start(
                    out=xt[:, c, :],
                    in_=xb[c, g * tokg:(g + 1) * tokg, :],
                    transpose=True,
                )
            for tt in range(tokg // 128):
                ps = ops.tile([128, osh], F32)
                for c in range(kc):
                    nc.tensor.matmul(
                        ps,
                        lhsT=xt[:, c, tt * 128:(tt + 1) * 128],
                        rhs=wbT[:, c, :],
                        start=(c == 0),
                        stop=(c == kc - 1),
                    )
                ob = otp.tile([128, osh], F32)
                nc.vector.tensor_tensor(out=ob, in0=ps, in1=bias_bc,
                                        op=mybir.AluOpType.add)
                row0 = g * tokg + tt * 128
                nc.sync.dma_start(out=out[row0:row0 + 128, :], in_=ob)


_NC_CACHE = {}


def _get_nc():
    if "nc" not in _NC_CACHE:
        _NC_CACHE["nc"] = build_kernel_nc()
    return _NC_CACHE["nc"]


def make_in_maps(x, weight, bias):
    """Host-side sharding: pre-tile bf16 activations, shard weight rows."""
    xb = x.reshape(TOK, DIN).astype(ml_dtypes.bfloat16)
    # [KC, TOK, 128]: contiguous 128-col chunks so each XBAR source slab
    # is a contiguous [tokg, 128] block.
    xb_t = np.ascontiguousarray(xb.reshape(TOK, KC, 128).transpose(1, 0, 2))
    in_maps = []
    for c in range(N_CORES):
        in_maps.append({
            "xb": xb_t,
            "w": np.ascontiguousarray(weight[c * OSH:(c + 1) * OSH]),
            "bias": np.ascontiguousarray(bias[c * OSH:(c + 1) * OSH]),
        })
    return in_maps


def kernel(x, weight, bias):
    x = np.asarray(x, dtype=np.float32)
    weight = np.asarray(weight, dtype=np.float32)
    bias = np.asarray(bias, dtype=np.float32)
    nc = _get_nc()
    in_maps = make_in_maps(x, weight, bias)
    res = run_bass_kernel_spmd(nc, in_maps, list(range(N_CORES)))
    out = np.concatenate(
        [res.results[c]["out"] for c in range(N_CORES)], axis=1)
    return out.reshape(B, S, DOUT).astype(np.float32)


# revision 5
# speedup vs baseline: 1.8200x; 1.8200x over previous
"""Trainium2 Bass kernel for nn_IrBinaryLinear (binarized linear layer).

Reference computation (fp32):
    w  = weight - mean(weight, axis=-1, keepdims=True)       # [out, in]
    s  = mean(|w|, axis=-1, keepdims=True)                   # [out, 1]
    wb = sign(w) * s                                         # [out, in]
    y  = x @ wb.T + bias                                     # [B, S, out]

Sharding: tensor-parallel over weight rows (out_features) across 8 cores.
Each core binarizes its own 512-row weight shard on device and transposes
it on the PE array. The (replicated) activations are fed already
contraction-major ([i-chunk, i-in-chunk, token] bf16, a host-side layout
choice) so each token group is one large contiguous-strided DMA at full
HBM bandwidth; the binarized weights are exactly +/-scale, which bf16
represents with only a 2^-9 relative rounding of the scale. PSUM
accumulates in fp32 and the bias is added on the vector engine before the
fp32 store.
"""

import numpy as np
import ml_dtypes

import concourse.bass as bass
import concourse.tile as tile
from concourse import bacc, mybir
from concourse.bass_utils import run_bass_kernel_spmd
from concourse.masks import make_identity

F32 = mybir.dt.float32
BF16 = mybir.dt.bfloat16

N_CORES = 8
B, S, DIN, DOUT = 4, 2048, 4096, 4096
TOK = B * S                    # 8192 tokens
OSH = DOUT // N_CORES          # 512 output rows per core
KC = DIN // 128                # 32 contraction chunks
TOKG = 512                     # tokens per XBAR-load group
RT = OSH // 128                # weight row tiles per core


def build_kernel_nc(tok=TOK, osh=OSH, tokg=TOKG, n_cores=N_CORES):
    """Build + compile the per-core Bass program (SPMD: same on all cores)."""
    kc = KC
    rt_n = osh // 128
    ntg = tok // tokg

    nc = bacc.Bacc("TRN2", target_bir_lowering=False, debug=False,
                   num_devices=n_cores)
    # x^T, chunk-tiled: xb[p, c, t] = x[t, c*128 + p] (bf16)
    xb_d = nc.dram_tensor("xb", [128, kc, tok], BF16, kind="ExternalInput")
    w_d = nc.dram_tensor("w", [osh, DIN], F32, kind="ExternalInput")
    bias_d = nc.dram_tensor("bias", [osh], F32, kind="ExternalInput")
    out_d = nc.dram_tensor("out", [tok, osh], F32, kind="ExternalOutput")

    with tile.TileContext(nc) as tc:
        _body(tc, nc, xb_d.ap(), w_d.ap(), bias_d.ap(), out_d.ap(),
              tok=tok, osh=osh, tokg=tokg, kc=kc, rt_n=rt_n, ntg=ntg)

    nc.compile()
    return nc


def _body(tc, nc, xb, w, bias, out, *, tok, osh, tokg, kc, rt_n, ntg):
    with (
        tc.tile_pool(name="consts", bufs=1) as consts,
        tc.tile_pool(name="wld", bufs=2) as wld,
        tc.tile_pool(name="wsg", bufs=2) as wsg,
        tc.tile_pool(name="wst", bufs=8) as wst,
        tc.tile_pool(name="wbtp", bufs=1) as wbtp,
        tc.tile_pool(name="tps", bufs=2, space="PSUM") as tps,
        tc.tile_pool(name="xtp", bufs=2) as xtp,
        tc.tile_pool(name="ops", bufs=4, space="PSUM") as ops,
        tc.tile_pool(name="otp", bufs=3) as otp,
    ):
        ident = consts.tile([128, 128], BF16)
        make_identity(nc, ident)

        # bias broadcast to all 128 partitions: [osh] -> [128, osh]
        bias_bc = consts.tile([128, osh], F32)
        bias_bcast_ap = bass.AP(
            tensor=bias.tensor, offset=bias.offset,
            ap=[[0, 128]] + list(bias.ap),
        )
        nc.gpsimd.dma_start(out=bias_bc, in_=bias_bcast_ap)

        # Binarized transposed weights, resident: [128(i), kc, osh] bf16
        wbT = wbtp.tile([128, kc, osh], BF16)

        for rt in range(rt_n):
            wt = wld.tile([128, DIN], F32)
            nc.sync.dma_start(out=wt, in_=w[rt * 128:(rt + 1) * 128, :])

            rs = wst.tile([128, 1], F32)
            nc.vector.tensor_reduce(out=rs, in_=wt, axis=mybir.AxisListType.X,
                                    op=mybir.AluOpType.add)
            nmean = wst.tile([128, 1], F32)
            nc.vector.tensor_scalar_mul(nmean, rs, -1.0 / DIN)

            # sgn = sign(w - mean)  (exact +/-1, bf16)
            sgn = wsg.tile([128, DIN], BF16)
            nc.scalar.sign(out=sgn, in_=wt, bias=nmean)

            # |w - mean| = (w + nmean) * sgn ; row-sum into asum
            asum = wst.tile([128, 1], F32)
            nc.vector.scalar_tensor_tensor(
                out=wt, in0=wt, scalar=nmean, in1=sgn,
                op0=mybir.AluOpType.add, op1=mybir.AluOpType.mult,
                accum_out=asum,
            )
            scale = wst.tile([128, 1], F32)
            nc.vector.tensor_scalar_mul(scale, asum, 1.0 / DIN)

            # wb row tile = sgn * scale (in place, bf16)
            nc.vector.tensor_scalar_mul(sgn, sgn, scale)

            # transpose [128(o), 128(i)] chunks onto PE -> wbT[:, c, o-range]
            for c in range(kc):
                pt = tps.tile([128, 128], BF16)
                nc.tensor.transpose(pt, sgn[:, c * 128:(c + 1) * 128], ident)
                nc.vector.tensor_copy(
                    out=wbT[:, c, rt * 128:(rt + 1) * 128], in_=pt)

        # main loop: stream x through XBAR transpose, matmul, bias, store
        for g in range(ntg):
            xt = xtp.tile([128, kc, tokg], BF16)
            nc.sync.dma_start(out=xt, in_=xb[:, :, g * tokg:(g + 1) * tokg])
            for tt in range(tokg // 128):
                ps = ops.tile([128, osh], F32)
                for c in range(kc):
                    nc.tensor.matmul(
                        ps,
                        lhsT=xt[:, c, tt * 128:(tt + 1) * 128],
                        rhs=wbT[:, c, :],
                        start=(c == 0),
                        stop=(c == kc - 1),
                    )
                ob = otp.tile([128, osh], F32)
                nc.vector.tensor_tensor(out=ob, in0=ps, in1=bias_bc,
                                        op=mybir.AluOpType.add)
                row0 = g * tokg + tt * 128
                nc.sync.dma_start(out=out[row0:row0 + 128, :], in_=ob)


_NC_CACHE = {}


def _get_nc():
    if "nc" not in _NC_CACHE:
        _NC_CACHE["nc"] = build_kernel_nc()
    return _NC_CACHE["nc"]


def make_in_maps(x, weight, bias):
    """Host-side sharding: pre-tile bf16 activations, shard weight rows."""
    xb = x.reshape(TOK, DIN).astype(ml_dtypes.bfloat16)
    # [128, KC, TOK]: xb_t[p, c, t] = x[t, c*128+p] — contraction on the
    # partition axis, 1KB-contiguous token runs for efficient DMA.
    xb_t = np.ascontiguousarray(xb.reshape(TOK, KC, 128).transpose(2, 1, 0))
    in_maps = []
    for c in range(N_CORES):
        in_maps.append({
            "xb": xb_t,
            "w": np.ascontiguousarray(weight[c * OSH:(c + 1) * OSH]),
            "bias": np.ascontiguousarray(bias[c * OSH:(c + 1) * OSH]),
        })
    return in_maps


def kernel(x, weight, bias):
    x = np.asarray(x, dtype=np.float32)
    weight = np.asarray(weight, dtype=np.float32)
    bias = np.asarray(bias, dtype=np.float32)
    nc = _get_nc()
    in_maps = make_in_maps(x, weight, bias)
    res = run_bass_kernel_spmd(nc, in_maps, list(range(N_CORES)))
    out = np.concatenate(
        [res.results[c]["out"] for c in range(N_CORES)], axis=1)
    return out.reshape(B, S, DOUT).astype(np.float32)


# revision 8
# speedup vs baseline: 1.8351x; 1.0083x over previous
"""Trainium2 Bass kernel for nn_IrBinaryLinear (binarized linear layer).

Reference computation (fp32):
    w  = weight - mean(weight, axis=-1, keepdims=True)       # [out, in]
    s  = mean(|w|, axis=-1, keepdims=True)                   # [out, 1]
    wb = sign(w) * s                                         # [out, in]
    y  = x @ wb.T + bias                                     # [B, S, out]

Sharding: tensor-parallel over weight rows (out_features) across 8 cores.
Each core binarizes its own 512-row weight shard on device and transposes
it on the PE array. The (replicated) activations are fed already
contraction-major ([i-chunk, i-in-chunk, token] bf16, a host-side layout
choice) so each token group is one large contiguous-strided DMA at full
HBM bandwidth; the binarized weights are exactly +/-scale, which bf16
represents with only a 2^-9 relative rounding of the scale. PSUM
accumulates in fp32 and the bias is added on the vector engine before the
fp32 store.
"""

import numpy as np
import ml_dtypes

import concourse.bass as bass
import concourse.tile as tile
from concourse import bacc, mybir
from concourse.bass_utils import run_bass_kernel_spmd
from concourse.masks import make_identity

F32 = mybir.dt.float32
BF16 = mybir.dt.bfloat16

N_CORES = 8
B, S, DIN, DOUT = 4, 2048, 4096, 4096
TOK = B * S                    # 8192 tokens
OSH = DOUT // N_CORES          # 512 output rows per core
KC = DIN // 128                # 32 contraction chunks
TOKG = 512                     # tokens per XBAR-load group
RT = OSH // 128                # weight row tiles per core


def build_kernel_nc(tok=TOK, osh=OSH, tokg=TOKG, n_cores=N_CORES):
    """Build + compile the per-core Bass program (SPMD: same on all cores)."""
    kc = KC
    rt_n = osh // 128
    ntg = tok // tokg

    nc = bacc.Bacc("TRN2", target_bir_lowering=False, debug=False,
                   num_devices=n_cores)
    # x^T, chunk-tiled: xb[p, c, t] = x[t, c*128 + p] (bf16)
    xb_d = nc.dram_tensor("xb", [128, kc, tok], BF16, kind="ExternalInput")
    w_d = nc.dram_tensor("w", [osh, DIN], F32, kind="ExternalInput")
    bias_d = nc.dram_tensor("bias", [osh], F32, kind="ExternalInput")
    out_d = nc.dram_tensor("out", [tok, osh], F32, kind="ExternalOutput")

    with tile.TileContext(nc) as tc:
        _body(tc, nc, xb_d.ap(), w_d.ap(), bias_d.ap(), out_d.ap(),
              tok=tok, osh=osh, tokg=tokg, kc=kc, rt_n=rt_n, ntg=ntg)

    nc.compile()
    return nc


def _body(tc, nc, xb, w, bias, out, *, tok, osh, tokg, kc, rt_n, ntg):
    with (
        tc.tile_pool(name="consts", bufs=1) as consts,
        tc.tile_pool(name="wld", bufs=2) as wld,
        tc.tile_pool(name="wsg", bufs=2) as wsg,
        tc.tile_pool(name="wst", bufs=8) as wst,
        tc.tile_pool(name="wbtp", bufs=1) as wbtp,
        tc.tile_pool(name="tps", bufs=2, space="PSUM") as tps,
        tc.tile_pool(name="xtp", bufs=3) as xtp,
        tc.tile_pool(name="ops", bufs=6, space="PSUM") as ops,
        tc.tile_pool(name="otp", bufs=3) as otp,
    ):
        ident = consts.tile([128, 128], BF16)
        make_identity(nc, ident)

        # bias broadcast to all 128 partitions: [osh] -> [128, osh]
        bias_bc = consts.tile([128, osh], F32)
        bias_bcast_ap = bass.AP(
            tensor=bias.tensor, offset=bias.offset,
            ap=[[0, 128]] + list(bias.ap),
        )
        nc.gpsimd.dma_start(out=bias_bc, in_=bias_bcast_ap)

        # Binarized transposed weights, resident: [128(i), kc, osh] bf16
        wbT = wbtp.tile([128, kc, osh], BF16)

        for rt in range(rt_n):
            wt = wld.tile([128, DIN], F32)
            nc.sync.dma_start(out=wt, in_=w[rt * 128:(rt + 1) * 128, :])

            rs = wst.tile([128, 1], F32)
            nc.vector.tensor_reduce(out=rs, in_=wt, axis=mybir.AxisListType.X,
                                    op=mybir.AluOpType.add)
            nmean = wst.tile([128, 1], F32)
            nc.vector.tensor_scalar_mul(nmean, rs, -1.0 / DIN)

            # sgn = sign(w - mean)  (exact +/-1, bf16)
            sgn = wsg.tile([128, DIN], BF16)
            nc.scalar.sign(out=sgn, in_=wt, bias=nmean)

            # |w - mean| = (w + nmean) * sgn ; row-sum into asum
            asum = wst.tile([128, 1], F32)
            nc.vector.scalar_tensor_tensor(
                out=wt, in0=wt, scalar=nmean, in1=sgn,
                op0=mybir.AluOpType.add, op1=mybir.AluOpType.mult,
                accum_out=asum,
            )
            scale = wst.tile([128, 1], F32)
            nc.vector.tensor_scalar_mul(scale, asum, 1.0 / DIN)

            # wb row tile = sgn * scale (in place, bf16)
            nc.vector.tensor_scalar_mul(sgn, sgn, scale)

            # transpose [128(o), 128(i)] chunks onto PE -> wbT[:, c, o-range]
            for c in range(kc):
                pt = tps.tile([128, 128], BF16)
                nc.tensor.transpose(pt, sgn[:, c * 128:(c + 1) * 128], ident)
                nc.vector.tensor_copy(
                    out=wbT[:, c, rt * 128:(rt + 1) * 128], in_=pt)

        # main loop: stream x through XBAR transpose, matmul, bias, store
        for g in range(ntg):
            xt = xtp.tile([128, kc, tokg], BF16)
            nc.sync.dma_start(out=xt, in_=xb[:, :, g * tokg:(g + 1) * tokg])
            for tt in range(tokg // 128):
                ps = ops.tile([128, osh], F32)
                for c in range(kc):
                    nc.tensor.matmul(
                        ps,
                        lhsT=xt[:, c, tt * 128:(tt + 1) * 128],
                        rhs=wbT[:, c, :],
                        start=(c == 0),
                        stop=(c == kc - 1),
                    )
                ob = otp.tile([128, osh], F32)
                nc.vector.tensor_tensor(out=ob, in0=ps, in1=bias_bc,
                                        op=mybir.AluOpType.add)
                row0 = g * tokg + tt * 128
                nc.sync.dma_start(out=out[row0:row0 + 128, :], in_=ob)


_NC_CACHE = {}


def _get_nc():
    if "nc" not in _NC_CACHE:
        _NC_CACHE["nc"] = build_kernel_nc()
    return _NC_CACHE["nc"]


def make_in_maps(x, weight, bias):
    """Host-side sharding: pre-tile bf16 activations, shard weight rows."""
    xb = x.reshape(TOK, DIN).astype(ml_dtypes.bfloat16)
    # [128, KC, TOK]: xb_t[p, c, t] = x[t, c*128+p] — contraction on the
    # partition axis, 1KB-contiguous token runs for efficient DMA.
    xb_t = np.ascontiguousarray(xb.reshape(TOK, KC, 128).transpose(2, 1, 0))
    in_maps = []
    for c in range(N_CORES):
        in_maps.append({
            "xb": xb_t,
            "w": np.ascontiguousarray(weight[c * OSH:(c + 1) * OSH]),
            "bias": np.ascontiguousarray(bias[c * OSH:(c + 1) * OSH]),
        })
    return in_maps


def kernel(x, weight, bias):
    x = np.asarray(x, dtype=np.float32)
    weight = np.asarray(weight, dtype=np.float32)
    bias = np.asarray(bias, dtype=np.float32)
    nc = _get_nc()
    in_maps = make_in_maps(x, weight, bias)
    res = run_bass_kernel_spmd(nc, in_maps, list(range(N_CORES)))
    out = np.concatenate(
        [res.results[c]["out"] for c in range(N_CORES)], axis=1)
    return out.reshape(B, S, DOUT).astype(np.float32)


# revision 9
# speedup vs baseline: 1.8676x; 1.0177x over previous
"""Trainium2 Bass kernel for nn_IrBinaryLinear (binarized linear layer).

Reference computation (fp32):
    w  = weight - mean(weight, axis=-1, keepdims=True)       # [out, in]
    s  = mean(|w|, axis=-1, keepdims=True)                   # [out, 1]
    wb = sign(w) * s                                         # [out, in]
    y  = x @ wb.T + bias                                     # [B, S, out]

Sharding: tensor-parallel over weight rows (out_features) across 8 cores.
Each core binarizes its own 512-row weight shard on device and transposes
it on the PE array. The (replicated) activations are fed already
contraction-major ([i-chunk, i-in-chunk, token] bf16, a host-side layout
choice) so each token group is one large contiguous-strided DMA at full
HBM bandwidth; the binarized weights are exactly +/-scale, which bf16
represents with only a 2^-9 relative rounding of the scale. PSUM
accumulates in fp32 and the bias is added on the vector engine before the
fp32 store.
"""

import numpy as np
import ml_dtypes

import concourse.bass as bass
import concourse.tile as tile
from concourse import bacc, mybir
from concourse.bass_utils import run_bass_kernel_spmd
from concourse.masks import make_identity

F32 = mybir.dt.float32
BF16 = mybir.dt.bfloat16

N_CORES = 8
B, S, DIN, DOUT = 4, 2048, 4096, 4096
TOK = B * S                    # 8192 tokens
OSH = DOUT // N_CORES          # 512 output rows per core
KC = DIN // 128                # 32 contraction chunks
TOKG = 512                     # tokens per XBAR-load group
RT = OSH // 128                # weight row tiles per core


def build_kernel_nc(tok=TOK, osh=OSH, tokg=TOKG, n_cores=N_CORES):
    """Build + compile the per-core Bass program (SPMD: same on all cores)."""
    kc = KC
    rt_n = osh // 128
    ntg = tok // tokg

    nc = bacc.Bacc("TRN2", target_bir_lowering=False, debug=False,
                   num_devices=n_cores)
    # x^T, chunk-tiled: xb[p, c, t] = x[t, c*128 + p] (bf16)
    xb_d = nc.dram_tensor("xb", [128, kc, tok], BF16, kind="ExternalInput")
    w_d = nc.dram_tensor("w", [osh, DIN], F32, kind="ExternalInput")
    bias_d = nc.dram_tensor("bias", [osh], F32, kind="ExternalInput")
    out_d = nc.dram_tensor("out", [tok, osh], F32, kind="ExternalOutput")

    with tile.TileContext(nc) as tc:
        _body(tc, nc, xb_d.ap(), w_d.ap(), bias_d.ap(), out_d.ap(),
              tok=tok, osh=osh, tokg=tokg, kc=kc, rt_n=rt_n, ntg=ntg)

    nc.compile()
    return nc


def _body(tc, nc, xb, w, bias, out, *, tok, osh, tokg, kc, rt_n, ntg):
    with (
        tc.tile_pool(name="consts", bufs=1) as consts,
        tc.tile_pool(name="wld", bufs=2) as wld,
        tc.tile_pool(name="wsg", bufs=2) as wsg,
        tc.tile_pool(name="wst", bufs=8) as wst,
        tc.tile_pool(name="wbtp", bufs=1) as wbtp,
        tc.tile_pool(name="tps", bufs=2, space="PSUM") as tps,
        tc.tile_pool(name="xtp", bufs=3) as xtp,
        tc.tile_pool(name="ops", bufs=6, space="PSUM") as ops,
        tc.tile_pool(name="otp", bufs=3) as otp,
    ):
        ident = consts.tile([128, 128], BF16)
        make_identity(nc, ident)

        # bias broadcast to all 128 partitions: [osh] -> [128, osh]
        bias_bc = consts.tile([128, osh], F32)
        bias_bcast_ap = bass.AP(
            tensor=bias.tensor, offset=bias.offset,
            ap=[[0, 128]] + list(bias.ap),
        )
        nc.gpsimd.dma_start(out=bias_bc, in_=bias_bcast_ap)

        # Binarized transposed weights, resident: [128(i), kc, osh] bf16
        wbT = wbtp.tile([128, kc, osh], BF16)

        for rt in range(rt_n):
            wt = wld.tile([128, DIN], F32)
            nc.sync.dma_start(out=wt, in_=w[rt * 128:(rt + 1) * 128, :])

            # row-sum on ACT (Identity + accum_out) — keeps the big reduce
            # off the DVE critical path; the full-size out is a scratch
            # write into the sgn buffer, overwritten by sign() below.
            sgn = wsg.tile([128, DIN], BF16)
            rs = wst.tile([128, 1], F32)
            nc.scalar.activation(out=sgn, in_=wt,
                                 func=mybir.ActivationFunctionType.Identity,
                                 accum_out=rs)
            nmean = wst.tile([128, 1], F32)
            nc.vector.tensor_scalar_mul(nmean, rs, -1.0 / DIN)

            # sgn = sign(w - mean)  (exact +/-1, bf16)
            nc.scalar.sign(out=sgn, in_=wt, bias=nmean)

            # |w - mean| = (w + nmean) * sgn ; row-sum into asum
            asum = wst.tile([128, 1], F32)
            nc.vector.scalar_tensor_tensor(
                out=wt, in0=wt, scalar=nmean, in1=sgn,
                op0=mybir.AluOpType.add, op1=mybir.AluOpType.mult,
                accum_out=asum,
            )
            scale = wst.tile([128, 1], F32)
            nc.vector.tensor_scalar_mul(scale, asum, 1.0 / DIN)

            # wb row tile = sgn * scale (in place, bf16)
            nc.vector.tensor_scalar_mul(sgn, sgn, scale)

            # transpose [128(o), 128(i)] chunks onto PE -> wbT[:, c, o-range]
            for c in range(kc):
                pt = tps.tile([128, 128], BF16)
                nc.tensor.transpose(pt, sgn[:, c * 128:(c + 1) * 128], ident)
                nc.vector.tensor_copy(
                    out=wbT[:, c, rt * 128:(rt + 1) * 128], in_=pt)

        # main loop: stream x through XBAR transpose, matmul, bias, store
        for g in range(ntg):
            xt = xtp.tile([128, kc, tokg], BF16)
            nc.sync.dma_start(out=xt, in_=xb[:, :, g * tokg:(g + 1) * tokg])
            for tt in range(tokg // 128):
                ps = ops.tile([128, osh], F32)
                for c in range(kc):
                    nc.tensor.matmul(
                        ps,
                        lhsT=xt[:, c, tt * 128:(tt + 1) * 128],
                        rhs=wbT[:, c, :],
                        start=(c == 0),
                        stop=(c == kc - 1),
                    )
                ob = otp.tile([128, osh], F32)
                nc.vector.tensor_tensor(out=ob, in0=ps, in1=bias_bc,
                                        op=mybir.AluOpType.add)
                row0 = g * tokg + tt * 128
                nc.sync.dma_start(out=out[row0:row0 + 128, :], in_=ob)


_NC_CACHE = {}


def _get_nc():
    if "nc" not in _NC_CACHE:
        _NC_CACHE["nc"] = build_kernel_nc()
    return _NC_CACHE["nc"]


def make_in_maps(x, weight, bias):
    """Host-side sharding: pre-tile bf16 activations, shard weight rows."""
    xb = x.reshape(TOK, DIN).astype(ml_dtypes.bfloat16)
    # [128, KC, TOK]: xb_t[p, c, t] = x[t, c*128+p] — contraction on the
    # partition axis, 1KB-contiguous token runs for efficient DMA.
    xb_t = np.ascontiguousarray(xb.reshape(TOK, KC, 128).transpose(2, 1, 0))
    in_maps = []
    for c in range(N_CORES):
        in_maps.append({
            "xb": xb_t,
            "w": np.ascontiguousarray(weight[c * OSH:(c + 1) * OSH]),
            "bias": np.ascontiguousarray(bias[c * OSH:(c + 1) * OSH]),
        })
    return in_maps


def kernel(x, weight, bias):
    x = np.asarray(x, dtype=np.float32)
    weight = np.asarray(weight, dtype=np.float32)
    bias = np.asarray(bias, dtype=np.float32)
    nc = _get_nc()
    in_maps = make_in_maps(x, weight, bias)
    res = run_bass_kernel_spmd(nc, in_maps, list(range(N_CORES)))
    out = np.concatenate(
        [res.results[c]["out"] for c in range(N_CORES)], axis=1)
    return out.reshape(B, S, DOUT).astype(np.float32)
